# revision 1
# baseline (speedup 1.0000x reference)
"""MemoryCrossAttention Trainium2 Bass kernel.

8-core data-parallel over query rows: core c handles batch c//2, row-half
c%2 (2048 rows). K/V come from the 256 memory tokens, computed per core.
All matmuls run as float32r (full PE rate at N>=512, ~1e-4 rel precision).
RMSNorm is computed on-chip via a squares pass + ones-matmul partition
reduction; softmax mask folds into the exp bias (per-partition, scoresT
layout); the softmax denominator is a ones-matmul over probsT with the
reciprocal fused into the attention-output eviction.
"""
from concourse.bass_utils import run_bass_kernel_spmd


from contextlib import ExitStack

import concourse.bass as bass
import concourse.tile as tile
from concourse import mybir

F32 = mybir.dt.float32
F32R = mybir.dt.float32r
BF16 = mybir.dt.bfloat16
P = 128


def build(nc, H, NH, R, M, eps=1e-6, phases=4):
    HD = 128
    assert H == NH * HD
    KT = H // P           # contraction tiles
    LQ = R // 512         # 512-wide l chunks
    NHTP = NH // 2        # head pairs (Q/G/O weight streaming)
    MT = M // P           # memory-token partition tiles (2)
    KH = min(8, NH)       # heads per K-proj psum group
    NKG = NH // KH        # K-proj head groups
    KGW = KH * P          # K-proj weight tile width
    NVC = max(1, (NH * HD) // 512)  # V d-chunks of 512
    scale = HD ** -0.5

    xT = nc.dram_tensor("xT", [H, R], F32R, kind="ExternalInput")
    memT = nc.dram_tensor("memT", [H, M], F32R, kind="ExternalInput")
    maskb = nc.dram_tensor("maskb", [P, MT], F32, kind="ExternalInput")
    wqT = nc.dram_tensor("wqT", [NHTP, KT, P, 256], F32R, kind="ExternalInput")
    wgT = nc.dram_tensor("wgT", [NHTP, KT, P, 256], F32R, kind="ExternalInput")
    woT = nc.dram_tensor("woT", [NHTP, KT, P, 256], F32R, kind="ExternalInput")
    wkT = nc.dram_tensor("wkT", [NKG, KT, P, KGW], F32R, kind="ExternalInput")
    wvT = nc.dram_tensor("wvT", [NVC, KT, P, 512], F32R, kind="ExternalInput")
    outT = nc.dram_tensor("outT", [H, R], F32, kind="ExternalOutput")

    with tile.TileContext(nc) as tc, ExitStack() as ctx:
        dram = ctx.enter_context(tc.tile_pool(name="dram", bufs=1, space="DRAM"))
        qspill = dram.tile([H, R], F32R)
        gspill = dram.tile([H, R], F32)
        aspill = dram.tile([H, R], F32R)
        s_scr = dram.tile([R], F32)
        rd_scr = dram.tile([NH, R], F32)

        const = ctx.enter_context(tc.tile_pool(name="const", bufs=1))
        ones_f32 = const.tile([P, 1], F32)
        nc.vector.memset(ones_f32, 1.0)
        ones_sb = const.tile([P, 1], F32R)
        nc.vector.tensor_copy(ones_sb, ones_f32)
        eps_sb = const.tile([1, 1], F32)
        nc.vector.memset(eps_sb, eps)
        mask_sb = const.tile([P, MT], F32)
        nc.sync.dma_start(out=mask_sb, in_=maskb[:])

        # persistent: K/V stay for phases B-C
        kvpool = ctx.enter_context(tc.tile_pool(name="kv", bufs=1))
        kT_big = kvpool.tile([P, NH, M], F32R)    # [d, h, m]
        vmd_big = kvpool.tile([P, MT, H], F32R)   # [m, mt, d_full]

        with tc.tile_pool(name="x", bufs=1) as xpool:
            x_big = xpool.tile([P, KT, R], F32R)  # xT tiles; becomes xnT in place

            # ================= Phase A: load x, rmsnorm =================
            for kt in range(KT):
                nc.sync.dma_start(out=x_big[:, kt, :],
                                  in_=xT[kt * P:(kt + 1) * P, :])

            with tc.tile_pool(name="x2", bufs=2) as x2p, \
                 tc.tile_pool(name="ssqp", bufs=1, space="PSUM") as ssqp, \
                 tc.tile_pool(name="sp", bufs=1) as sp:
                ssq = [ssqp.tile([1, 512], F32, name=f"ssq{i}") for i in range(LQ)]
                for kt in range(KT):
                    x2 = x2p.tile([P, R], F32R)
                    nc.vector.tensor_mul(x2, x_big[:, kt, :], x_big[:, kt, :])
                    for lq in range(LQ):
                        nc.tensor.matmul(
                            ssq[lq], ones_sb, x2[:, lq * 512:(lq + 1) * 512],
                            start=(kt == 0), stop=(kt == KT - 1))
                s_sb = sp.tile([1, R], F32)
                rs_sb = sp.tile([1, R], F32)
                for lq in range(LQ):
                    nc.scalar.activation(
                        s_sb[:, lq * 512:(lq + 1) * 512], ssq[lq],
                        mybir.ActivationFunctionType.Sqrt,
                        bias=eps_sb, scale=1.0 / H)
                nc.vector.reciprocal(rs_sb, s_sb)
                nc.sync.dma_start(out=s_scr[:], in_=rs_sb[0:1, :])

            with tc.tile_pool(name="sbc", bufs=1) as sbcp:
                s_bc = sbcp.tile([P, R], F32)
                s_ap = s_scr[:]
                nc.sync.dma_start(
                    out=s_bc,
                    in_=bass.AP(tensor=s_ap.tensor, offset=s_ap.offset,
                                ap=[[0, P]] + s_ap.ap))
                for kt in range(KT):
                    nc.vector.tensor_mul(x_big[:, kt, :], x_big[:, kt, :], s_bc)

            # ============= Phase B1: K/V from memory tokens =============
            with tc.tile_pool(name="memp", bufs=1) as memp:
                mem_big = memp.tile([P, KT, M], F32R)
                for kt in range(KT):
                    nc.sync.dma_start(out=mem_big[:, kt, :],
                                      in_=memT[kt * P:(kt + 1) * P, :])

                with tc.tile_pool(name="wkst", bufs=3) as wkst, \
                     tc.tile_pool(name="kps", bufs=1, space="PSUM") as kps:
                    for kg in range(NKG):
                        kpsum = [kps.tile([P, M], F32, name=f"kpsum{i}")
                                 for i in range(KH)]
                        for kt in range(KT):
                            wk_t = wkst.tile([P, KGW], F32R)
                            nc.sync.dma_start(out=wk_t, in_=wkT[kg, kt])
                            for hh in range(KH):
                                nc.tensor.matmul(
                                    kpsum[hh], wk_t[:, hh * P:(hh + 1) * P],
                                    mem_big[:, kt, :],
                                    start=(kt == 0), stop=(kt == KT - 1))
                        for hh in range(KH):
                            nc.vector.tensor_copy(kT_big[:, kg * KH + hh, :],
                                                  kpsum[hh])

                with tc.tile_pool(name="wvst", bufs=3) as wvst, \
                     tc.tile_pool(name="vps", bufs=1, space="PSUM") as vps:
                    for dc in range(NVC):
                        vpsum = [vps.tile([P, 512], F32, name=f"vpsum{i}")
                                 for i in range(MT)]
                        for kt in range(KT):
                            wv_t = wvst.tile([P, 512], F32R)
                            nc.sync.dma_start(out=wv_t, in_=wvT[dc, kt])
                            for mt in range(MT):
                                nc.tensor.matmul(
                                    vpsum[mt],
                                    mem_big[:, kt, mt * P:(mt + 1) * P],
                                    wv_t,
                                    start=(kt == 0), stop=(kt == KT - 1))
                        for mt in range(MT):
                            nc.vector.tensor_copy(
                                vmd_big[:, mt, dc * 512:(dc + 1) * 512], vpsum[mt])

            # ============= Phase B2: Q and gate projections =============
            if phases < 2:
                return
            with tc.tile_pool(name="wqst", bufs=3) as wqst, \
                 tc.tile_pool(name="qps", bufs=1, space="PSUM") as qps, \
                 tc.tile_pool(name="qbuf", bufs=4) as qbufp, \
                 tc.tile_pool(name="gbuf", bufs=4) as gbufp:
                for htp in range(NHTP):
                    qpsum = [[qps.tile([P, 512], F32, name=f"qpsum{i}_{j}")
                              for j in range(LQ)] for i in range(2)]
                    for kt in range(KT):
                        wq_t = wqst.tile([P, 256], F32R)
                        nc.sync.dma_start(out=wq_t, in_=wqT[htp, kt])
                        for h2 in range(2):
                            for lq in range(LQ):
                                nc.tensor.matmul(
                                    qpsum[h2][lq], wq_t[:, h2 * P:(h2 + 1) * P],
                                    x_big[:, kt, lq * 512:(lq + 1) * 512],
                                    start=(kt == 0), stop=(kt == KT - 1))
                    for h2 in range(2):
                        ho = htp * 2 + h2
                        for lq in range(LQ):
                            qb = qbufp.tile([P, 512], F32R, name="qb")
                            nc.scalar.copy(qb, qpsum[h2][lq])
                            nc.sync.dma_start(
                                out=qspill[ho * P:(ho + 1) * P,
                                           lq * 512:(lq + 1) * 512],
                                in_=qb)

                for htp in range(NHTP):
                    gpsum = [[qps.tile([P, 512], F32, name=f"qpsum{i}_{j}")
                              for j in range(LQ)] for i in range(2)]
                    for kt in range(KT):
                        wg_t = wqst.tile([P, 256], F32R)
                        nc.sync.dma_start(out=wg_t, in_=wgT[htp, kt])
                        for h2 in range(2):
                            for lq in range(LQ):
                                nc.tensor.matmul(
                                    gpsum[h2][lq], wg_t[:, h2 * P:(h2 + 1) * P],
                                    x_big[:, kt, lq * 512:(lq + 1) * 512],
                                    start=(kt == 0), stop=(kt == KT - 1))
                    for h2 in range(2):
                        ho = htp * 2 + h2
                        for lq in range(LQ):
                            gb = gbufp.tile([P, 512], F32, name="gb")
                            nc.scalar.activation(
                                gb, gpsum[h2][lq],
                                mybir.ActivationFunctionType.Sigmoid)
                            nc.sync.dma_start(
                                out=gspill[ho * P:(ho + 1) * P,
                                           lq * 512:(lq + 1) * 512],
                                in_=gb)

        # ================= Phase C: attention per head =================
        if phases < 3:
            return
        with tc.tile_pool(name="qh", bufs=2) as qhp, \
             tc.tile_pool(name="probs", bufs=2) as probsp, \
             tc.tile_pool(name="rden", bufs=2) as rdenp, \
             tc.tile_pool(name="asb", bufs=2) as asbp, \
             tc.tile_pool(name="sps", bufs=4, space="PSUM") as sps, \
             tc.tile_pool(name="dps", bufs=2, space="PSUM") as dps, \
             tc.tile_pool(name="aps", bufs=2, space="PSUM") as aps:
            for h in range(NH):
                qh = qhp.tile([P, R], F32R, name="qh")
                nc.sync.dma_start(out=qh, in_=qspill[h * P:(h + 1) * P, :])

                probs = probsp.tile([P, MT, R], F32R, name="probs")
                for mt in range(MT):
                    for lq in range(LQ):
                        spsum = sps.tile([P, 512], F32, name="spsum")
                        nc.tensor.matmul(
                            spsum, kT_big[:, h, mt * P:(mt + 1) * P],
                            qh[:, lq * 512:(lq + 1) * 512],
                            start=True, stop=True)
                        nc.scalar.activation(
                            probs[:, mt, lq * 512:(lq + 1) * 512], spsum,
                            mybir.ActivationFunctionType.Exp,
                            bias=mask_sb[:, mt:mt + 1], scale=scale)

                rden = rdenp.tile([1, R], F32, name="rden")
                for lq in range(LQ):
                    dpsum = dps.tile([1, 512], F32, name="dpsum")
                    for mt in range(MT):
                        nc.tensor.matmul(
                            dpsum, ones_sb,
                            probs[:, mt, lq * 512:(lq + 1) * 512],
                            start=(mt == 0), stop=(mt == MT - 1))
                    nc.vector.reciprocal(rden[:, lq * 512:(lq + 1) * 512], dpsum)
                nc.sync.dma_start(out=rd_scr[h:h + 1, :], in_=rden[0:1, :])

                rden_bc = rdenp.tile([P, R], F32, name="rden_bc")
                rd_ap = rd_scr[h, :]
                nc.sync.dma_start(
                    out=rden_bc,
                    in_=bass.AP(tensor=rd_ap.tensor, offset=rd_ap.offset,
                                ap=[[0, P]] + rd_ap.ap))

                attn_sb = asbp.tile([P, R], F32R, name="attn_sb")
                for lq in range(LQ):
                    apsum = aps.tile([P, 512], F32, name="apsum")
                    for mt in range(MT):
                        nc.tensor.matmul(
                            apsum, vmd_big[:, mt, h * P:(h + 1) * P],
                            probs[:, mt, lq * 512:(lq + 1) * 512],
                            start=(mt == 0), stop=(mt == MT - 1))
                    nc.vector.tensor_mul(
                        attn_sb[:, lq * 512:(lq + 1) * 512], apsum,
                        rden_bc[:, lq * 512:(lq + 1) * 512])
                nc.sync.dma_start(out=aspill[h * P:(h + 1) * P, :], in_=attn_sb)

        # ================= Phase D: O-proj + gate =================
        if phases < 4:
            return
        with tc.tile_pool(name="at", bufs=1) as atp, \
             tc.tile_pool(name="wost", bufs=3) as wost, \
             tc.tile_pool(name="gin", bufs=2) as ginp, \
             tc.tile_pool(name="osb", bufs=2) as osbp, \
             tc.tile_pool(name="ops", bufs=1, space="PSUM") as ops:
            at_big = atp.tile([P, KT, R], F32R)
            for kt in range(KT):
                nc.sync.dma_start(out=at_big[:, kt, :],
                                  in_=aspill[kt * P:(kt + 1) * P, :])
            for htp in range(NHTP):
                opsum = [[ops.tile([P, 512], F32, name=f"opsum{i}_{j}")
                          for j in range(LQ)] for i in range(2)]
                for kt in range(KT):
                    wo_t = wost.tile([P, 256], F32R)
                    nc.sync.dma_start(out=wo_t, in_=woT[htp, kt])
                    for h2 in range(2):
                        for lq in range(LQ):
                            nc.tensor.matmul(
                                opsum[h2][lq], wo_t[:, h2 * P:(h2 + 1) * P],
                                at_big[:, kt, lq * 512:(lq + 1) * 512],
                                start=(kt == 0), stop=(kt == KT - 1))
                for h2 in range(2):
                    ho = htp * 2 + h2
                    g_in = ginp.tile([P, R], F32, name="g_in")
                    nc.sync.dma_start(out=g_in,
                                      in_=gspill[ho * P:(ho + 1) * P, :])
                    o_sb = osbp.tile([P, R], F32, name="o_sb")
                    for lq in range(LQ):
                        nc.vector.tensor_mul(
                            o_sb[:, lq * 512:(lq + 1) * 512], opsum[h2][lq],
                            g_in[:, lq * 512:(lq + 1) * 512])
                    nc.sync.dma_start(out=outT[ho * P:(ho + 1) * P, :], in_=o_sb)

    nc.compile()
    return nc


def prep_inputs(hs_slice, mem_b, mask_b, norm_w, wq, wk, wv, wo, wg, NH):
    """Host-side prep for one core. hs_slice [R, H], mem_b [M, H], mask_b [M]."""
    import numpy as np
    H = hs_slice.shape[1]
    M = mem_b.shape[0]
    P = 128
    KT = H // P
    KH = min(8, NH)
    KGW = KH * P

    def tile_w(wT, width):
        # wT [H, H] -> [H//width, KT, 128, width]
        n = wT.shape[1] // width
        return np.ascontiguousarray(
            wT.reshape(KT, P, n, width).transpose(2, 0, 1, 3))

    wq_n = (wq * norm_w[None, :]).T.astype(np.float32)   # [in, out]
    wg_n = (wg * norm_w[None, :]).T.astype(np.float32)
    wo_t = wo.T.astype(np.float32)
    wk_t = wk.T.astype(np.float32)
    wv_t = wv.T.astype(np.float32)

    maskb = np.where(mask_b, 0.0, -50.0).astype(np.float32)
    maskb = np.ascontiguousarray(maskb.reshape(M // P, P).T)  # [128, MT]

    return {
        "xT": np.ascontiguousarray(hs_slice.T.astype(np.float32)),
        "memT": np.ascontiguousarray(mem_b.T.astype(np.float32)),
        "maskb": maskb,
        "wqT": tile_w(wq_n, 256),
        "wgT": tile_w(wg_n, 256),
        "woT": tile_w(wo_t, 256),
        "wkT": tile_w(wk_t, KGW),
        "wvT": tile_w(wv_t, 512),
    }


import numpy as np

_H, _NH, _HD, _M = 2048, 16, 128, 256
_B, _L = 4, 4096
_RPC = 2048          # rows per core
_NCORES = 8
_EPS = 1e-6

_nc_cache = [None]


def _prep_core(hs_slice, mem_b, mask_b, shared):
    inp = dict(shared)
    inp["xT"] = np.ascontiguousarray(hs_slice.T)
    inp["memT"] = np.ascontiguousarray(mem_b.T)
    maskb = np.where(mask_b, 0.0, -50.0).astype(np.float32)
    inp["maskb"] = np.ascontiguousarray(maskb.reshape(_M // 128, 128).T)
    return inp


def _tile_w(wT, width):
    KT = wT.shape[0] // 128
    n = wT.shape[1] // width
    return np.ascontiguousarray(
        wT.reshape(KT, 128, n, width).transpose(2, 0, 1, 3))


def kernel(hidden_states, memory_tokens, memory_mask, norm_w,
           wq, wk, wv, wo, wg):
    import concourse.bacc as bacc

    hs = np.asarray(hidden_states, dtype=np.float32)
    mem = np.asarray(memory_tokens, dtype=np.float32)
    mask = np.asarray(memory_mask)
    norm_w = np.asarray(norm_w, dtype=np.float32)

    wq_n = (np.asarray(wq, dtype=np.float32) * norm_w[None, :]).T
    wg_n = (np.asarray(wg, dtype=np.float32) * norm_w[None, :]).T
    shared = {
        "wqT": _tile_w(np.ascontiguousarray(wq_n), 256),
        "wgT": _tile_w(np.ascontiguousarray(wg_n), 256),
        "woT": _tile_w(np.ascontiguousarray(np.asarray(wo, dtype=np.float32).T), 256),
        "wkT": _tile_w(np.ascontiguousarray(np.asarray(wk, dtype=np.float32).T), 1024),
        "wvT": _tile_w(np.ascontiguousarray(np.asarray(wv, dtype=np.float32).T), 512),
    }

    in_maps = []
    for c in range(_NCORES):
        b, half = c // 2, c % 2
        hs_slice = hs[b, half * _RPC:(half + 1) * _RPC, :]
        in_maps.append(_prep_core(hs_slice, mem[b], mask[b], shared))

    if _nc_cache[0] is None:
        nc = bacc.Bacc(None, target_bir_lowering=False, debug=False)
        build(nc, _H, _NH, _RPC, _M, eps=_EPS)
        _nc_cache[0] = nc
    nc = _nc_cache[0]

    import os
    trace = os.environ.get("KERNEL_TRACE") == "1"
    res = run_bass_kernel_spmd(nc, in_maps, core_ids=list(range(_NCORES)),
                               trace=trace)
    kernel.last_result = res

    out = np.empty((_B, _L, _H), dtype=np.float32)
    for c in range(_NCORES):
        b, half = c // 2, c % 2
        out[b, half * _RPC:(half + 1) * _RPC, :] = res.results[c]["outT"].T
    return out



# revision 22
# speedup vs baseline: 1.5677x; 1.5677x over previous
"""MemoryCrossAttention Trainium2 Bass kernel (fp16 full-rate rewrite).

8-core data-parallel over query rows: core c handles batch c//2, row-half
c%2 (2048 rows). All matmuls run in fp16 (full PE rate; fp32r is
throttled to half rate on TRN2 hardware). Weights and x are cast to fp16
on the host. Everything stays resident in SBUF (no DRAM spills):
Q-projection is fused with attention per head, and the gate projection is
fused with the O-projection (sigmoid applied at PSUM eviction). Softmax
denominators accumulate into one PSUM bank at partition offsets
{0,32,64,96} (valid matmul tile positions), get a fast reciprocal, and
are broadcast to 128 partitions with a stride-0 SBUF DMA.
"""
from concourse.bass_utils import run_bass_kernel_spmd


from contextlib import ExitStack

import concourse.bass as bass
import concourse.tile as tile
from concourse import mybir

F32 = mybir.dt.float32
F16 = mybir.dt.float16
P = 128
LN256 = 5.545177444479562


def _bcast_ap(row_ap, n=512):
    """[1, n] AP -> [128, n] partition-broadcast AP (stride 0)."""
    return bass.AP(tensor=row_ap.tensor, offset=row_ap.offset,
                   ap=[[0, P]] + row_ap.ap)


def build(nc, H, NH, R, M, eps=1e-6, phases=4):
    HD = 128
    assert H == NH * HD
    KT = H // P           # contraction tiles (16)
    LQ = R // 512         # 512-wide row chunks (4)
    MT = M // P           # memory-token partition tiles (2)
    HQ = 4                # heads per Q-weight chunk
    NHG = NH // HQ        # 4
    KH = 8                # heads per K-proj psum group
    NKG = NH // KH        # 2
    NVC = H // 512        # V d-chunks (4)
    scale = HD ** -0.5

    xhT = nc.dram_tensor("xhT", [H, R], F16, kind="ExternalInput")
    memT = nc.dram_tensor("memT", [H, M], F16, kind="ExternalInput")
    maskb = nc.dram_tensor("maskb", [P, MT], F32, kind="ExternalInput")
    wqT = nc.dram_tensor("wqT", [NHG, KT, P, HQ * P], F16, kind="ExternalInput")
    wgT = nc.dram_tensor("wgT", [NH // 2, KT, P, 2 * P], F16,
                         kind="ExternalInput")
    woT = nc.dram_tensor("woT", [NH // 2, KT, P, 2 * P], F16,
                         kind="ExternalInput")
    wkT = nc.dram_tensor("wkT", [NKG, KT, P, KH * P], F16, kind="ExternalInput")
    wvT = nc.dram_tensor("wvT", [NVC, KT, P, 512], F16, kind="ExternalInput")
    outT = nc.dram_tensor("outT", [H, R], F32, kind="ExternalOutput")

    with tile.TileContext(nc) as tc, ExitStack() as ctx:
        dram = ctx.enter_context(tc.tile_pool(name="dram", bufs=1, space="DRAM"))
        rs_scr = dram.tile([LQ, 512], F16)
        rd_scr = dram.tile([NH, LQ, 512], F16)

        const = ctx.enter_context(tc.tile_pool(name="const", bufs=1))
        ones_t = const.tile([P, 1], F16)
        nc.vector.memset(ones_t, 1.0)
        eps_sb = const.tile([P, 1], F32)
        nc.vector.memset(eps_sb, eps)
        mask_sb = const.tile([P, MT], F32)
        nc.sync.dma_start(out=mask_sb, in_=maskb[:])

        # persistent activations
        xpool = ctx.enter_context(tc.tile_pool(name="x", bufs=1))
        xh = xpool.tile([P, KT, R], F16)          # x, then xn in place
        kvpool = ctx.enter_context(tc.tile_pool(name="kv", bufs=1))
        kT_all = kvpool.tile([P, NH, M], F16)     # [d, h, m]
        vmd = kvpool.tile([P, MT, H], F16)        # [m, mt, d_full]
        apool = ctx.enter_context(tc.tile_pool(name="attn", bufs=1))
        attn_all = apool.tile([P, NH, R], F16)    # [d, h, rows]

        # ================= Phase A: load x, rmsnorm =================
        for kt in range(KT):
            nc.sync.dma_start(out=xh[:, kt, :], in_=xhT[kt * P:(kt + 1) * P, :])

        with tc.tile_pool(name="x2", bufs=2) as x2p, \
             tc.tile_pool(name="ssqp", bufs=1, space="PSUM") as ssqp, \
             tc.tile_pool(name="rsp", bufs=1) as rsp:
            ssq = [ssqp.tile([1, 512], F32, name=f"ssq{j}")
                   for j in range(LQ)]
            for kt in range(KT):
                x2 = x2p.tile([P, R], F16)
                nc.vector.tensor_mul(x2, xh[:, kt, :], xh[:, kt, :])
                for lq in range(LQ):
                    nc.tensor.matmul(
                        ssq[lq], ones_t,
                        x2[:, lq * 512:(lq + 1) * 512],
                        start=(kt == 0), stop=(kt == KT - 1))
            s_sb = rsp.tile([1, LQ, 512], F32)
            rs_sb = rsp.tile([1, LQ, 512], F32)
            rsh_sb = rsp.tile([1, LQ, 512], F16)
            rsb = rsp.tile([P, R], F16)
            for lq in range(LQ):
                nc.scalar.activation(
                    s_sb[:, lq, :], ssq[lq],
                    mybir.ActivationFunctionType.Sqrt,
                    bias=eps_sb[0:1, :], scale=1.0 / H)
                nc.vector.reciprocal_approx_fast(rs_sb[:, lq, :],
                                                 s_sb[:, lq, :])
                nc.vector.tensor_copy(rsh_sb[:, lq, :], rs_sb[:, lq, :])
                nc.sync.dma_start(out=rs_scr[lq, :], in_=rsh_sb[:, lq, :])
                nc.sync.dma_start(
                    out=rsb[:, lq * 512:(lq + 1) * 512],
                    in_=_bcast_ap(rs_scr[lq, :]))
            for kt in range(KT):
                for lq in range(LQ):
                    c = slice(lq * 512, (lq + 1) * 512)
                    nc.vector.tensor_mul(xh[:, kt, c], xh[:, kt, c], rsb[:, c])

        # ============= Phase B: K/V from memory tokens =============
        with tc.tile_pool(name="memp", bufs=1) as memp:
            memh = memp.tile([P, KT, M], F16)
            for kt in range(KT):
                nc.sync.dma_start(out=memh[:, kt, :],
                                  in_=memT[kt * P:(kt + 1) * P, :])

            with tc.tile_pool(name="wkst", bufs=3) as wkst, \
                 tc.tile_pool(name="kps", bufs=1, space="PSUM") as kps:
                for kg in range(NKG):
                    kpsum = [kps.tile([P, M], F32, name=f"kpsum{i}")
                             for i in range(KH)]
                    for kt in range(KT):
                        wk_t = wkst.tile([P, KH * P], F16)
                        nc.sync.dma_start(out=wk_t, in_=wkT[kg, kt])
                        for hh in range(KH):
                            nc.tensor.matmul(
                                kpsum[hh], wk_t[:, hh * P:(hh + 1) * P],
                                memh[:, kt, :],
                                start=(kt == 0), stop=(kt == KT - 1))
                    for hh in range(KH):
                        nc.scalar.copy(kT_all[:, kg * KH + hh, :], kpsum[hh])

            with tc.tile_pool(name="wvst", bufs=3) as wvst, \
                 tc.tile_pool(name="vps", bufs=2, space="PSUM") as vps:
                for dc in range(NVC):
                    vpsum = [vps.tile([P, 512], F32, name=f"vpsum{i}")
                             for i in range(MT)]
                    for kt in range(KT):
                        wv_t = wvst.tile([P, 512], F16)
                        nc.sync.dma_start(out=wv_t, in_=wvT[dc, kt])
                        for mt in range(MT):
                            nc.tensor.matmul(
                                vpsum[mt], memh[:, kt, mt * P:(mt + 1) * P],
                                wv_t, start=(kt == 0), stop=(kt == KT - 1))
                    for mt in range(MT):
                        nc.scalar.copy(
                            vmd[:, mt, dc * 512:(dc + 1) * 512], vpsum[mt])

        # ========== Phase C: Q proj + attention, fused per head ==========
        if phases < 3:
            return
        with tc.tile_pool(name="wqst", bufs=2) as wqst, \
             tc.tile_pool(name="qh", bufs=2) as qhp, \
             tc.tile_pool(name="probs", bufs=1) as probsp, \
             tc.tile_pool(name="rden", bufs=1) as rdenp, \
             tc.tile_pool(name="rrow", bufs=2) as rrowp, \
             tc.tile_pool(name="qps", bufs=2, space="PSUM") as qps, \
             tc.tile_pool(name="dpps", bufs=2, space="PSUM") as dpps, \
             tc.tile_pool(name="tmpps", bufs=2, space="PSUM") as tmpps:
            for hg in range(NHG):
                wq_t = wqst.tile([P, KT, HQ * P], F16, name="wq_t")
                for kt in range(KT):
                    nc.sync.dma_start(out=wq_t[:, kt, :], in_=wqT[hg, kt])
                for hh in range(HQ):
                    h = hg * HQ + hh
                    # Q projection for head h
                    qh = qhp.tile([P, R], F16, name="qh")
                    for lq in range(LQ):
                        qpsum = qps.tile([P, 512], F32, name="qpsum")
                        for kt in range(KT):
                            nc.tensor.matmul(
                                qpsum, wq_t[:, kt, hh * P:(hh + 1) * P],
                                xh[:, kt, lq * 512:(lq + 1) * 512],
                                start=(kt == 0), stop=(kt == KT - 1))
                        nc.scalar.copy(qh[:, lq * 512:(lq + 1) * 512], qpsum)
                    # scores -> probs (exp with mask bias, /256 folded in)
                    probs = probsp.tile([P, MT, R], F16, name="probs")
                    for mt in range(MT):
                        for lq in range(LQ):
                            sp = tmpps.tile([P, 512], F32, name="sp")
                            nc.tensor.matmul(
                                sp, kT_all[:, h, mt * P:(mt + 1) * P],
                                qh[:, lq * 512:(lq + 1) * 512],
                                start=True, stop=True)
                            nc.scalar.activation(
                                probs[:, mt, lq * 512:(lq + 1) * 512], sp,
                                mybir.ActivationFunctionType.Exp,
                                bias=mask_sb[:, mt:mt + 1], scale=scale)
                    # denominators: one [1,512] bank per row chunk
                    rdenb = rdenp.tile([P, R], F16, name="rdenb")
                    for lq in range(LQ):
                        dpb = dpps.tile([1, 512], F32, name="dpb")
                        for mt in range(MT):
                            nc.tensor.matmul(
                                dpb, ones_t,
                                probs[:, mt, lq * 512:(lq + 1) * 512],
                                start=(mt == 0), stop=(mt == MT - 1))
                        rr = rrowp.tile([1, 512], F32, name="rr")
                        rh = rrowp.tile([1, 512], F16, name="rh")
                        nc.vector.reciprocal_approx_fast(rr, dpb)
                        nc.vector.tensor_copy(rh, rr)
                        nc.sync.dma_start(out=rd_scr[h, lq, :], in_=rh)
                        nc.sync.dma_start(
                            out=rdenb[:, lq * 512:(lq + 1) * 512],
                            in_=_bcast_ap(rd_scr[h, lq, :]))
                    # attention output, normalized at eviction
                    for lq in range(LQ):
                        ap_ = tmpps.tile([P, 512], F32, name="ap")
                        for mt in range(MT):
                            nc.tensor.matmul(
                                ap_, vmd[:, mt, h * P:(h + 1) * P],
                                probs[:, mt, lq * 512:(lq + 1) * 512],
                                start=(mt == 0), stop=(mt == MT - 1))
                        c = slice(lq * 512, (lq + 1) * 512)
                        nc.vector.tensor_mul(attn_all[:, h, c], ap_,
                                             rdenb[:, c])

        # ============== Phase D: O proj + gate, fused ==============
        if phases < 4:
            return
        with tc.tile_pool(name="wost", bufs=2) as wost, \
             tc.tile_pool(name="wgst", bufs=2) as wgst, \
             tc.tile_pool(name="gs", bufs=4) as gsp, \
             tc.tile_pool(name="osb", bufs=2) as osbp, \
             tc.tile_pool(name="ops", bufs=2, space="PSUM") as ops:
            for hog in range(NH // 2):
                wo_t = wost.tile([P, KT, 2 * P], F16, name="wo_t")
                wg_t = wgst.tile([P, KT, 2 * P], F16, name="wg_t")
                for kt in range(KT):
                    nc.sync.dma_start(out=wo_t[:, kt, :], in_=woT[hog, kt])
                    nc.sync.dma_start(out=wg_t[:, kt, :], in_=wgT[hog, kt])
                for hh in range(2):
                    ho = hog * 2 + hh
                    for lqp in range(2):
                        op2 = [ops.tile([P, 512], F32, name=f"op{j}")
                               for j in range(2)]
                        gp2 = [ops.tile([P, 512], F32, name=f"gp{j}")
                               for j in range(2)]
                        for kt in range(KT):
                            for j in range(2):
                                c = slice(lqp * 1024 + j * 512,
                                          lqp * 1024 + (j + 1) * 512)
                                nc.tensor.matmul(
                                    op2[j], wo_t[:, kt, hh * P:(hh + 1) * P],
                                    attn_all[:, kt, c],
                                    start=(kt == 0), stop=(kt == KT - 1))
                                nc.tensor.matmul(
                                    gp2[j], wg_t[:, kt, hh * P:(hh + 1) * P],
                                    xh[:, kt, c],
                                    start=(kt == 0), stop=(kt == KT - 1))
                        o_sb = osbp.tile([P, 1024], F32, name="o_sb")
                        for j in range(2):
                            gs = gsp.tile([P, 512], F16, name="gs")
                            nc.scalar.activation(
                                gs, gp2[j],
                                mybir.ActivationFunctionType.Sigmoid)
                            nc.vector.tensor_mul(
                                o_sb[:, j * 512:(j + 1) * 512], op2[j], gs)
                        nc.sync.dma_start(
                            out=outT[ho * P:(ho + 1) * P,
                                     lqp * 1024:(lqp + 1) * 1024],
                            in_=o_sb)

    nc.compile()
    return nc


import numpy as np

_H, _NH, _HD, _M = 2048, 16, 128, 256
_B, _L = 4, 4096
_RPC = 2048          # rows per core
_NCORES = 8
_EPS = 1e-6

_nc_cache = [None]


def _tile_w(wT, width):
    KT = wT.shape[0] // 128
    n = wT.shape[1] // width
    return np.ascontiguousarray(
        wT.reshape(KT, 128, n, width).transpose(2, 0, 1, 3).astype(np.float16))


def _prep_core(hs_slice, mem_b, mask_b, shared):
    inp = dict(shared)
    inp["xhT"] = np.ascontiguousarray(hs_slice.T.astype(np.float16))
    inp["memT"] = np.ascontiguousarray(mem_b.T.astype(np.float16))
    maskb = np.where(mask_b, -LN256, -50.0).astype(np.float32)
    inp["maskb"] = np.ascontiguousarray(maskb.reshape(_M // 128, 128).T)
    return inp


def kernel(hidden_states, memory_tokens, memory_mask, norm_w,
           wq, wk, wv, wo, wg):
    import concourse.bacc as bacc

    hs = np.asarray(hidden_states, dtype=np.float32)
    mem = np.asarray(memory_tokens, dtype=np.float32)
    mask = np.asarray(memory_mask)
    norm_w = np.asarray(norm_w, dtype=np.float32)

    wq_n = (np.asarray(wq, dtype=np.float32) * norm_w[None, :]).T
    wg_n = (np.asarray(wg, dtype=np.float32) * norm_w[None, :]).T
    shared = {
        "wqT": _tile_w(np.ascontiguousarray(wq_n), 512),
        "wgT": _tile_w(np.ascontiguousarray(wg_n), 256),
        "woT": _tile_w(np.ascontiguousarray(np.asarray(wo, dtype=np.float32).T), 256),
        "wkT": _tile_w(np.ascontiguousarray(np.asarray(wk, dtype=np.float32).T), 1024),
        "wvT": _tile_w(np.ascontiguousarray(np.asarray(wv, dtype=np.float32).T), 512),
    }

    in_maps = []
    for c in range(_NCORES):
        b, half = c // 2, c % 2
        hs_slice = hs[b, half * _RPC:(half + 1) * _RPC, :]
        in_maps.append(_prep_core(hs_slice, mem[b], mask[b], shared))

    if _nc_cache[0] is None:
        nc = bacc.Bacc(None, target_bir_lowering=False, debug=False)
        build(nc, _H, _NH, _RPC, _M, eps=_EPS)
        _nc_cache[0] = nc
    nc = _nc_cache[0]

    import os
    trace = os.environ.get("KERNEL_TRACE") == "1"
    res = run_bass_kernel_spmd(nc, in_maps, core_ids=list(range(_NCORES)),
                               trace=trace)
    kernel.last_result = res

    out = np.empty((_B, _L, _H), dtype=np.float32)
    for c in range(_NCORES):
        b, half = c // 2, c % 2
        out[b, half * _RPC:(half + 1) * _RPC, :] = res.results[c]["outT"].T
    return out


# revision 25
# speedup vs baseline: 1.6460x; 1.0500x over previous
"""MemoryCrossAttention Trainium2 Bass kernel (fp16 full-rate rewrite).

8-core data-parallel over query rows: core c handles batch c//2, row-half
c%2 (2048 rows). All matmuls run in fp16 (full PE rate; fp32r is
throttled to half rate on TRN2 hardware). Weights and x are cast to fp16
on the host with p-major tiling so every weight group loads in one
large-line DMA. Everything stays resident in SBUF (no DRAM spills).
Emission order keeps the PE fed: K/V projections run first (they only
need the small memory-token tiles), the RMSNorm square-sum accumulates
behind them, and phase C is software-pipelined (Q-projection of head h+1
is emitted between the scores and denominator of head h so the exp/
reciprocal latency never stalls the PE). The gate projection is fused
into the O-projection with sigmoid applied at PSUM eviction.
"""
from concourse.bass_utils import run_bass_kernel_spmd


from contextlib import ExitStack

import concourse.bass as bass
import concourse.tile as tile
from concourse import mybir

F32 = mybir.dt.float32
F16 = mybir.dt.float16
P = 128
LN256 = 5.545177444479562


def _bcast_ap(row_ap):
    """[1, n] AP -> [128, n] partition-broadcast AP (stride 0)."""
    return bass.AP(tensor=row_ap.tensor, offset=row_ap.offset,
                   ap=[[0, P]] + row_ap.ap)


def build(nc, H, NH, R, M, eps=1e-6):
    HD = 128
    assert H == NH * HD
    KT = H // P           # contraction tiles (16)
    LQ = R // 512         # 512-wide row chunks (4)
    MT = M // P           # memory-token partition tiles (2)
    NHG = NH // 2         # 2-head weight groups (8)
    KH = 8                # heads per K-proj psum group
    NKG = NH // KH        # 2
    NVC = H // 512        # V d-chunks (4)
    scale = HD ** -0.5

    xhT = nc.dram_tensor("xhT", [H, R], F16, kind="ExternalInput")
    memT = nc.dram_tensor("memT", [P, KT, M], F16, kind="ExternalInput")
    maskb = nc.dram_tensor("maskb", [P, MT], F32, kind="ExternalInput")
    wqT = nc.dram_tensor("wqT", [NHG, P, KT, 2 * P], F16, kind="ExternalInput")
    wgT = nc.dram_tensor("wgT", [NHG, P, KT, 2 * P], F16, kind="ExternalInput")
    woT = nc.dram_tensor("woT", [NHG, P, KT, 2 * P], F16, kind="ExternalInput")
    wkT = nc.dram_tensor("wkT", [NKG, P, KT, KH * P], F16, kind="ExternalInput")
    wvT = nc.dram_tensor("wvT", [NVC, P, KT, 512], F16, kind="ExternalInput")
    outT = nc.dram_tensor("outT", [H, R], F32, kind="ExternalOutput")

    with tile.TileContext(nc) as tc, ExitStack() as ctx:
        dram = ctx.enter_context(tc.tile_pool(name="dram", bufs=1, space="DRAM"))
        rs_scr = dram.tile([LQ, 512], F16)
        rd_scr = dram.tile([NH, LQ, 512], F16)

        const = ctx.enter_context(tc.tile_pool(name="const", bufs=1))
        ones_t = const.tile([P, 1], F16)
        nc.vector.memset(ones_t, 1.0)
        eps_sb = const.tile([P, 1], F32)
        nc.vector.memset(eps_sb, eps)
        mask_sb = const.tile([P, MT], F32)
        nc.sync.dma_start(out=mask_sb, in_=maskb[:])

        # persistent activations
        xpool = ctx.enter_context(tc.tile_pool(name="x", bufs=1))
        xh = xpool.tile([P, KT, R], F16)          # x, then xn in place
        kvpool = ctx.enter_context(tc.tile_pool(name="kv", bufs=1))
        kT_all = kvpool.tile([P, NH, M], F16)     # [d, h, m]
        vmd = kvpool.tile([P, MT, H], F16)        # [m, mt, d_full]
        apool = ctx.enter_context(tc.tile_pool(name="attn", bufs=1))
        attn_all = apool.tile([P, NH, R], F16)    # [d, h, rows]

        # ============= Phase B: K/V from memory tokens =============
        with tc.tile_pool(name="memp", bufs=1) as memp:
            memh = memp.tile([P, KT, M], F16)
            nc.sync.dma_start(out=memh, in_=memT[:])
            for kt in range(KT):
                nc.sync.dma_start(out=xh[:, kt, :],
                                  in_=xhT[kt * P:(kt + 1) * P, :])

            with tc.tile_pool(name="wkst", bufs=2) as wkst, \
                 tc.tile_pool(name="kps", bufs=1, space="PSUM") as kps:
                for kg in range(NKG):
                    kpsum = [kps.tile([P, M], F32, name=f"kpsum{i}")
                             for i in range(KH)]
                    for half in range(2):
                        wk_t = wkst.tile([P, KT // 2, KH * P], F16)
                        nc.sync.dma_start(
                            out=wk_t, in_=wkT[kg][:, half * (KT // 2):
                                                  (half + 1) * (KT // 2), :])
                        for k8 in range(KT // 2):
                            kt = half * (KT // 2) + k8
                            for hh in range(KH):
                                nc.tensor.matmul(
                                    kpsum[hh],
                                    wk_t[:, k8, hh * P:(hh + 1) * P],
                                    memh[:, kt, :],
                                    start=(kt == 0), stop=(kt == KT - 1))
                    for hh in range(KH):
                        nc.scalar.copy(kT_all[:, kg * KH + hh, :], kpsum[hh])

            with tc.tile_pool(name="wvst", bufs=2) as wvst, \
                 tc.tile_pool(name="vps", bufs=2, space="PSUM") as vps:
                for dc in range(NVC):
                    wv_t = wvst.tile([P, KT, 512], F16)
                    nc.sync.dma_start(out=wv_t, in_=wvT[dc])
                    vpsum = [vps.tile([P, 512], F32, name=f"vpsum{i}")
                             for i in range(MT)]
                    for kt in range(KT):
                        for mt in range(MT):
                            nc.tensor.matmul(
                                vpsum[mt], memh[:, kt, mt * P:(mt + 1) * P],
                                wv_t[:, kt, :],
                                start=(kt == 0), stop=(kt == KT - 1))
                    for mt in range(MT):
                        nc.scalar.copy(
                            vmd[:, mt, dc * 512:(dc + 1) * 512], vpsum[mt])

        # ================= Phase A: rmsnorm (behind K/V) =================
        with tc.tile_pool(name="x2", bufs=2) as x2p, \
             tc.tile_pool(name="ssqp", bufs=1, space="PSUM") as ssqp, \
             tc.tile_pool(name="rsp", bufs=1) as rsp:
            ssq = [ssqp.tile([1, 512], F32, name=f"ssq{j}")
                   for j in range(LQ)]
            for kt in range(KT):
                x2 = x2p.tile([P, R], F16)
                nc.vector.tensor_mul(x2, xh[:, kt, :], xh[:, kt, :])
                for lq in range(LQ):
                    nc.tensor.matmul(
                        ssq[lq], ones_t,
                        x2[:, lq * 512:(lq + 1) * 512],
                        start=(kt == 0), stop=(kt == KT - 1))
            s_sb = rsp.tile([1, LQ, 512], F32)
            rs_sb = rsp.tile([1, LQ, 512], F32)
            rsh_sb = rsp.tile([1, LQ, 512], F16)
            rsb = rsp.tile([P, R], F16)
            for lq in range(LQ):
                nc.scalar.activation(
                    s_sb[:, lq, :], ssq[lq],
                    mybir.ActivationFunctionType.Sqrt,
                    bias=eps_sb[0:1, :], scale=1.0 / H)
                nc.vector.reciprocal_approx_fast(rs_sb[:, lq, :],
                                                 s_sb[:, lq, :])
                nc.vector.tensor_copy(rsh_sb[:, lq, :], rs_sb[:, lq, :])
                nc.sync.dma_start(out=rs_scr[lq, :], in_=rsh_sb[:, lq, :])
                nc.sync.dma_start(
                    out=rsb[:, lq * 512:(lq + 1) * 512],
                    in_=_bcast_ap(rs_scr[lq, :]))
            for lq in range(LQ):
                c = slice(lq * 512, (lq + 1) * 512)
                for kt in range(KT):
                    nc.vector.tensor_mul(xh[:, kt, c], xh[:, kt, c], rsb[:, c])

        # ====== preload O/gate group 0 so phase D starts instantly ======
        with tc.tile_pool(name="wod0", bufs=1) as w0p:
            wo0 = w0p.tile([P, KT, 2 * P], F16, name="wo0")
            wg0 = w0p.tile([P, KT, 2 * P], F16, name="wg0")
            nc.sync.dma_start(out=wo0, in_=woT[0])
            nc.sync.dma_start(out=wg0, in_=wgT[0])

            # ===== Phase C: Q proj + attention, software-pipelined =====
            with tc.tile_pool(name="wqst", bufs=2) as wqst, \
                 tc.tile_pool(name="qh", bufs=2) as qhp, \
                 tc.tile_pool(name="probs", bufs=1) as probsp, \
                 tc.tile_pool(name="rden", bufs=1) as rdenp, \
                 tc.tile_pool(name="rrow", bufs=2) as rrowp, \
                 tc.tile_pool(name="qps", bufs=2, space="PSUM") as qps, \
                 tc.tile_pool(name="dpps", bufs=2, space="PSUM") as dpps, \
                 tc.tile_pool(name="tmpps", bufs=2, space="PSUM") as tmpps:
                wq_tiles = {}

                def qproj(h):
                    hg, hh = divmod(h, 2)
                    if hh == 0:
                        wq_t = wqst.tile([P, KT, 2 * P], F16, name="wq_t")
                        nc.sync.dma_start(out=wq_t, in_=wqT[hg])
                        wq_tiles[hg] = wq_t
                    wq_t = wq_tiles[hg]
                    qh = qhp.tile([P, R], F16, name="qh")
                    for lq in range(LQ):
                        qpsum = qps.tile([P, 512], F32, name="qpsum")
                        for kt in range(KT):
                            nc.tensor.matmul(
                                qpsum, wq_t[:, kt, hh * P:(hh + 1) * P],
                                xh[:, kt, lq * 512:(lq + 1) * 512],
                                start=(kt == 0), stop=(kt == KT - 1))
                        nc.scalar.copy(qh[:, lq * 512:(lq + 1) * 512], qpsum)
                    return qh

                qh_cur = qproj(0)
                for h in range(NH):
                    # scores -> probs (exp with mask bias, /256 folded in)
                    probs = probsp.tile([P, MT, R], F16, name="probs")
                    for mt in range(MT):
                        for lq in range(LQ):
                            sp = tmpps.tile([P, 512], F32, name="sp")
                            nc.tensor.matmul(
                                sp, kT_all[:, h, mt * P:(mt + 1) * P],
                                qh_cur[:, lq * 512:(lq + 1) * 512],
                                start=True, stop=True)
                            nc.scalar.activation(
                                probs[:, mt, lq * 512:(lq + 1) * 512], sp,
                                mybir.ActivationFunctionType.Exp,
                                bias=mask_sb[:, mt:mt + 1], scale=scale)
                    # Q projection of the NEXT head hides exp/recip latency
                    if h + 1 < NH:
                        qh_cur = qproj(h + 1)
                    # denominators: one [1,512] bank per row chunk
                    rdenb = rdenp.tile([P, R], F16, name="rdenb")
                    for lq in range(LQ):
                        dpb = dpps.tile([1, 512], F32, name="dpb")
                        for mt in range(MT):
                            nc.tensor.matmul(
                                dpb, ones_t,
                                probs[:, mt, lq * 512:(lq + 1) * 512],
                                start=(mt == 0), stop=(mt == MT - 1))
                        rr = rrowp.tile([1, 512], F32, name="rr")
                        rh = rrowp.tile([1, 512], F16, name="rh")
                        nc.vector.reciprocal_approx_fast(rr, dpb)
                        nc.vector.tensor_copy(rh, rr)
                        nc.sync.dma_start(out=rd_scr[h, lq, :], in_=rh)
                        nc.sync.dma_start(
                            out=rdenb[:, lq * 512:(lq + 1) * 512],
                            in_=_bcast_ap(rd_scr[h, lq, :]))
                    # attention output, normalized at eviction
                    for lq in range(LQ):
                        ap_ = tmpps.tile([P, 512], F32, name="ap")
                        for mt in range(MT):
                            nc.tensor.matmul(
                                ap_, vmd[:, mt, h * P:(h + 1) * P],
                                probs[:, mt, lq * 512:(lq + 1) * 512],
                                start=(mt == 0), stop=(mt == MT - 1))
                        c = slice(lq * 512, (lq + 1) * 512)
                        nc.vector.tensor_mul(attn_all[:, h, c], ap_,
                                             rdenb[:, c])

            # ============== Phase D: O proj + gate, fused ==============
            with tc.tile_pool(name="wost", bufs=2) as wost, \
                 tc.tile_pool(name="wgst", bufs=2) as wgst, \
                 tc.tile_pool(name="gs", bufs=4) as gsp, \
                 tc.tile_pool(name="osb", bufs=2) as osbp, \
                 tc.tile_pool(name="ops", bufs=2, space="PSUM") as ops:
                for hog in range(NHG):
                    if hog == 0:
                        wo_t, wg_t = wo0, wg0
                    else:
                        wo_t = wost.tile([P, KT, 2 * P], F16, name="wo_t")
                        wg_t = wgst.tile([P, KT, 2 * P], F16, name="wg_t")
                        nc.sync.dma_start(out=wo_t, in_=woT[hog])
                        nc.sync.dma_start(out=wg_t, in_=wgT[hog])
                    for hh in range(2):
                        ho = hog * 2 + hh
                        for lqp in range(2):
                            op2 = [ops.tile([P, 512], F32, name=f"op{j}")
                                   for j in range(2)]
                            gp2 = [ops.tile([P, 512], F32, name=f"gp{j}")
                                   for j in range(2)]
                            for kt in range(KT):
                                for j in range(2):
                                    c = slice(lqp * 1024 + j * 512,
                                              lqp * 1024 + (j + 1) * 512)
                                    nc.tensor.matmul(
                                        op2[j],
                                        wo_t[:, kt, hh * P:(hh + 1) * P],
                                        attn_all[:, kt, c],
                                        start=(kt == 0), stop=(kt == KT - 1))
                                for j in range(2):
                                    c = slice(lqp * 1024 + j * 512,
                                              lqp * 1024 + (j + 1) * 512)
                                    nc.tensor.matmul(
                                        gp2[j],
                                        wg_t[:, kt, hh * P:(hh + 1) * P],
                                        xh[:, kt, c],
                                        start=(kt == 0), stop=(kt == KT - 1))
                            o_sb = osbp.tile([P, 1024], F32, name="o_sb")
                            for j in range(2):
                                gs = gsp.tile([P, 512], F16, name="gs")
                                nc.scalar.activation(
                                    gs, gp2[j],
                                    mybir.ActivationFunctionType.Sigmoid)
                                nc.vector.tensor_mul(
                                    o_sb[:, j * 512:(j + 1) * 512], op2[j], gs)
                            nc.sync.dma_start(
                                out=outT[ho * P:(ho + 1) * P,
                                         lqp * 1024:(lqp + 1) * 1024],
                                in_=o_sb)

    nc.compile()
    return nc


import numpy as np

_H, _NH, _HD, _M = 2048, 16, 128, 256
_B, _L = 4, 4096
_RPC = 2048          # rows per core
_NCORES = 8
_EPS = 1e-6

_nc_cache = [None]


def _tile_w(wT, width):
    """[in, out] f32 -> [n, P, KT, width] fp16, p-major for large DMA lines."""
    KT = wT.shape[0] // 128
    n = wT.shape[1] // width
    return np.ascontiguousarray(
        wT.reshape(KT, 128, n, width).transpose(2, 1, 0, 3).astype(np.float16))


def _prep_core(hs_slice, mem_b, mask_b, shared):
    inp = dict(shared)
    inp["xhT"] = np.ascontiguousarray(hs_slice.T.astype(np.float16))
    memt = mem_b.T.astype(np.float16)          # [H, M]
    inp["memT"] = np.ascontiguousarray(
        memt.reshape(_H // 128, 128, _M).transpose(1, 0, 2))
    maskb = np.where(mask_b, -LN256, -50.0).astype(np.float32)
    inp["maskb"] = np.ascontiguousarray(maskb.reshape(_M // 128, 128).T)
    return inp


def kernel(hidden_states, memory_tokens, memory_mask, norm_w,
           wq, wk, wv, wo, wg):
    import concourse.bacc as bacc

    hs = np.asarray(hidden_states, dtype=np.float32)
    mem = np.asarray(memory_tokens, dtype=np.float32)
    mask = np.asarray(memory_mask)
    norm_w = np.asarray(norm_w, dtype=np.float32)

    wq_n = (np.asarray(wq, dtype=np.float32) * norm_w[None, :]).T
    wg_n = (np.asarray(wg, dtype=np.float32) * norm_w[None, :]).T
    shared = {
        "wqT": _tile_w(np.ascontiguousarray(wq_n), 256),
        "wgT": _tile_w(np.ascontiguousarray(wg_n), 256),
        "woT": _tile_w(np.ascontiguousarray(np.asarray(wo, dtype=np.float32).T), 256),
        "wkT": _tile_w(np.ascontiguousarray(np.asarray(wk, dtype=np.float32).T), 1024),
        "wvT": _tile_w(np.ascontiguousarray(np.asarray(wv, dtype=np.float32).T), 512),
    }

    in_maps = []
    for c in range(_NCORES):
        b, half = c // 2, c % 2
        hs_slice = hs[b, half * _RPC:(half + 1) * _RPC, :]
        in_maps.append(_prep_core(hs_slice, mem[b], mask[b], shared))

    if _nc_cache[0] is None:
        nc = bacc.Bacc(None, target_bir_lowering=False, debug=False)
        build(nc, _H, _NH, _RPC, _M, eps=_EPS)
        _nc_cache[0] = nc
    nc = _nc_cache[0]

    import os
    trace = os.environ.get("KERNEL_TRACE") == "1"
    res = run_bass_kernel_spmd(nc, in_maps, core_ids=list(range(_NCORES)),
                               trace=trace)
    kernel.last_result = res

    out = np.empty((_B, _L, _H), dtype=np.float32)
    for c in range(_NCORES):
        b, half = c // 2, c % 2
        out[b, half * _RPC:(half + 1) * _RPC, :] = res.results[c]["outT"].T
    return out


# revision 29
# speedup vs baseline: 1.6608x; 1.0089x over previous
"""MemoryCrossAttention Trainium2 Bass kernel (fp16 full-rate rewrite).

8-core data-parallel over query rows: core c handles batch c//2, row-half
c%2 (2048 rows). All matmuls run in fp16 (full PE rate; fp32r is
throttled to half rate on TRN2 hardware). Weights and x are cast to fp16
on the host with p-major tiling so every weight group loads in one
large-line DMA. Everything stays resident in SBUF (no DRAM spills).
Emission order keeps the PE fed: K/V projections run first (they only
need the small memory-token tiles), the RMSNorm square-sum accumulates
behind them, and phase C is software-pipelined (Q-projection of head h+1
is emitted between the scores and denominator of head h so the exp/
reciprocal latency never stalls the PE). The gate projection is fused
into the O-projection with sigmoid applied at PSUM eviction.
"""
from concourse.bass_utils import run_bass_kernel_spmd


from contextlib import ExitStack

import concourse.bass as bass
import concourse.tile as tile
from concourse import mybir

F32 = mybir.dt.float32
F16 = mybir.dt.float16
P = 128
LN256 = 5.545177444479562


def _bcast_ap(row_ap):
    """[1, n] AP -> [128, n] partition-broadcast AP (stride 0)."""
    return bass.AP(tensor=row_ap.tensor, offset=row_ap.offset,
                   ap=[[0, P]] + row_ap.ap)


def build(nc, H, NH, R, M, eps=1e-6):
    HD = 128
    assert H == NH * HD
    KT = H // P           # contraction tiles (16)
    LQ = R // 512         # 512-wide row chunks (4)
    MT = M // P           # memory-token partition tiles (2)
    NHG = NH // 2         # 2-head weight groups (8)
    KH = 8                # heads per K-proj psum group
    NKG = NH // KH        # 2
    NVC = H // 512        # V d-chunks (4)
    scale = HD ** -0.5

    xhT = nc.dram_tensor("xhT", [H, R], F16, kind="ExternalInput")
    memT = nc.dram_tensor("memT", [P, KT, M], F16, kind="ExternalInput")
    maskb = nc.dram_tensor("maskb", [P, MT], F32, kind="ExternalInput")
    wqT = nc.dram_tensor("wqT", [NHG, P, KT, 2 * P], F16, kind="ExternalInput")
    wgT = nc.dram_tensor("wgT", [NHG, P, KT, 2 * P], F16, kind="ExternalInput")
    woT = nc.dram_tensor("woT", [NHG, P, KT, 2 * P], F16, kind="ExternalInput")
    wkT = nc.dram_tensor("wkT", [NKG, P, KT, KH * P], F16, kind="ExternalInput")
    wvT = nc.dram_tensor("wvT", [NVC, P, KT, 512], F16, kind="ExternalInput")
    outT = nc.dram_tensor("outT", [H, R], F32, kind="ExternalOutput")

    with tile.TileContext(nc) as tc, ExitStack() as ctx:
        dram = ctx.enter_context(tc.tile_pool(name="dram", bufs=1, space="DRAM"))
        rs_scr = dram.tile([LQ, 512], F16)
        rd_scr = dram.tile([NH, LQ, 512], F16)

        const = ctx.enter_context(tc.tile_pool(name="const", bufs=1))
        ones_t = const.tile([P, 1], F16)
        nc.vector.memset(ones_t, 1.0)
        eps_sb = const.tile([P, 1], F32)
        nc.vector.memset(eps_sb, eps)
        mask_sb = const.tile([P, MT], F32)
        nc.sync.dma_start(out=mask_sb, in_=maskb[:])

        # persistent activations
        xpool = ctx.enter_context(tc.tile_pool(name="x", bufs=1))
        xh = xpool.tile([P, KT, R], F16)          # x, then xn in place
        kvpool = ctx.enter_context(tc.tile_pool(name="kv", bufs=1))
        kT_all = kvpool.tile([P, NH, M], F16)     # [d, h, m]
        vmd = kvpool.tile([P, MT, H], F16)        # [m, mt, d_full]
        apool = ctx.enter_context(tc.tile_pool(name="attn", bufs=1))
        attn_all = apool.tile([P, NH, R], F16)    # [d, h, rows]

        # ============= Phase B: K/V from memory tokens =============
        # DMA issue order is the startup critical path: memory tokens and
        # K weights go first so the PE starts at ~5us; x tiles stream in
        # two batches between the K-weight chunks; V weights follow.
        with tc.tile_pool(name="memp", bufs=1) as memp:
            memh = memp.tile([P, KT, M], F16)
            nc.sync.dma_start(out=memh, in_=memT[:])

            with tc.tile_pool(name="wkst", bufs=3) as wkst, \
                 tc.tile_pool(name="kps", bufs=1, space="PSUM") as kps:
                KT2 = KT // 2
                wk_t = {}
                for kg, half in ((0, 0), (0, 1)):
                    wk_t[kg, half] = wkst.tile([P, KT2, KH * P], F16, name="wk_t")
                    nc.sync.dma_start(
                        out=wk_t[kg, half],
                        in_=wkT[kg][:, half * KT2:(half + 1) * KT2, :])
                for kt in range(KT // 2):
                    nc.sync.dma_start(out=xh[:, kt, :],
                                      in_=xhT[kt * P:(kt + 1) * P, :])
                wk_t[1, 0] = wkst.tile([P, KT2, KH * P], F16, name="wk_t")
                nc.sync.dma_start(out=wk_t[1, 0], in_=wkT[1][:, 0:KT2, :])
                for kt in range(KT // 2, KT):
                    nc.sync.dma_start(out=xh[:, kt, :],
                                      in_=xhT[kt * P:(kt + 1) * P, :])
                wk_t[1, 1] = wkst.tile([P, KT2, KH * P], F16, name="wk_t")
                nc.sync.dma_start(out=wk_t[1, 1], in_=wkT[1][:, KT2:KT, :])
                for kg in range(NKG):
                    kpsum = [kps.tile([P, M], F32, name=f"kpsum{i}")
                             for i in range(KH)]
                    for half in range(2):
                        for k8 in range(KT2):
                            kt = half * KT2 + k8
                            for hh in range(KH):
                                nc.tensor.matmul(
                                    kpsum[hh],
                                    wk_t[kg, half][:, k8, hh * P:(hh + 1) * P],
                                    memh[:, kt, :],
                                    start=(kt == 0), stop=(kt == KT - 1))
                    for hh in range(KH):
                        nc.scalar.copy(kT_all[:, kg * KH + hh, :], kpsum[hh])

            with tc.tile_pool(name="wvst", bufs=2) as wvst, \
                 tc.tile_pool(name="vps", bufs=2, space="PSUM") as vps:
                for dc in range(NVC):
                    wv_t = wvst.tile([P, KT, 512], F16)
                    nc.sync.dma_start(out=wv_t, in_=wvT[dc])
                    vpsum = [vps.tile([P, 512], F32, name=f"vpsum{i}")
                             for i in range(MT)]
                    for kt in range(KT):
                        for mt in range(MT):
                            nc.tensor.matmul(
                                vpsum[mt], memh[:, kt, mt * P:(mt + 1) * P],
                                wv_t[:, kt, :],
                                start=(kt == 0), stop=(kt == KT - 1))
                    for mt in range(MT):
                        nc.scalar.copy(
                            vmd[:, mt, dc * 512:(dc + 1) * 512], vpsum[mt])

        # ================= Phase A: rmsnorm (behind K/V) =================
        with tc.tile_pool(name="x2", bufs=2) as x2p, \
             tc.tile_pool(name="ssqp", bufs=1, space="PSUM") as ssqp, \
             tc.tile_pool(name="rsp", bufs=1) as rsp:
            ssq = [ssqp.tile([1, 512], F32, name=f"ssq{j}")
                   for j in range(LQ)]
            for kt in range(KT):
                x2 = x2p.tile([P, R], F16)
                nc.vector.tensor_mul(x2, xh[:, kt, :], xh[:, kt, :])
                for lq in range(LQ):
                    nc.tensor.matmul(
                        ssq[lq], ones_t,
                        x2[:, lq * 512:(lq + 1) * 512],
                        start=(kt == 0), stop=(kt == KT - 1))
            s_sb = rsp.tile([1, LQ, 512], F32)
            rs_sb = rsp.tile([1, LQ, 512], F32)
            rsh_sb = rsp.tile([1, LQ, 512], F16)
            rsb = rsp.tile([P, R], F16)
            for lq in range(LQ):
                nc.scalar.activation(
                    s_sb[:, lq, :], ssq[lq],
                    mybir.ActivationFunctionType.Sqrt,
                    bias=eps_sb[0:1, :], scale=1.0 / H)
                nc.vector.reciprocal_approx_fast(rs_sb[:, lq, :],
                                                 s_sb[:, lq, :])
                nc.vector.tensor_copy(rsh_sb[:, lq, :], rs_sb[:, lq, :])
                nc.sync.dma_start(out=rs_scr[lq, :], in_=rsh_sb[:, lq, :])
                nc.sync.dma_start(
                    out=rsb[:, lq * 512:(lq + 1) * 512],
                    in_=_bcast_ap(rs_scr[lq, :]))
            for lq in range(LQ):
                c = slice(lq * 512, (lq + 1) * 512)
                for kt in range(KT):
                    nc.vector.tensor_mul(xh[:, kt, c], xh[:, kt, c], rsb[:, c])

        # ====== preload O/gate group 0 so phase D starts instantly ======
        with tc.tile_pool(name="wod0", bufs=1) as w0p:
            wo0 = w0p.tile([P, KT, 2 * P], F16, name="wo0")
            wg0 = w0p.tile([P, KT, 2 * P], F16, name="wg0")

            # ===== Phase C: Q proj + attention, software-pipelined =====
            with tc.tile_pool(name="wqst", bufs=2) as wqst, \
                 tc.tile_pool(name="qh", bufs=2) as qhp, \
                 tc.tile_pool(name="probs", bufs=1) as probsp, \
                 tc.tile_pool(name="rden", bufs=1) as rdenp, \
                 tc.tile_pool(name="rrow", bufs=2) as rrowp, \
                 tc.tile_pool(name="qps", bufs=2, space="PSUM") as qps, \
                 tc.tile_pool(name="dpps", bufs=2, space="PSUM") as dpps, \
                 tc.tile_pool(name="tmpps", bufs=2, space="PSUM") as tmpps:
                wq_tiles = {}

                def qproj(h):
                    hg, hh = divmod(h, 2)
                    if hh == 0:
                        wq_t = wqst.tile([P, KT, 2 * P], F16, name="wq_t")
                        nc.sync.dma_start(out=wq_t, in_=wqT[hg])
                        wq_tiles[hg] = wq_t
                    wq_t = wq_tiles[hg]
                    qh = qhp.tile([P, R], F16, name="qh")
                    for lq in range(LQ):
                        qpsum = qps.tile([P, 512], F32, name="qpsum")
                        for kt in range(KT):
                            nc.tensor.matmul(
                                qpsum, wq_t[:, kt, hh * P:(hh + 1) * P],
                                xh[:, kt, lq * 512:(lq + 1) * 512],
                                start=(kt == 0), stop=(kt == KT - 1))
                        nc.scalar.copy(qh[:, lq * 512:(lq + 1) * 512], qpsum)
                    return qh

                qh_cur = qproj(0)
                for h in range(NH):
                    # scores -> probs (exp with mask bias, /256 folded in)
                    probs = probsp.tile([P, MT, R], F16, name="probs")
                    for mt in range(MT):
                        for lq in range(LQ):
                            sp = tmpps.tile([P, 512], F32, name="sp")
                            nc.tensor.matmul(
                                sp, kT_all[:, h, mt * P:(mt + 1) * P],
                                qh_cur[:, lq * 512:(lq + 1) * 512],
                                start=True, stop=True)
                            nc.scalar.activation(
                                probs[:, mt, lq * 512:(lq + 1) * 512], sp,
                                mybir.ActivationFunctionType.Exp,
                                bias=mask_sb[:, mt:mt + 1], scale=scale)
                    # Q projection of the NEXT head hides exp/recip latency
                    if h + 1 < NH:
                        qh_cur = qproj(h + 1)
                    if h == 1:
                        # D group-0 weights: issued here so they queue
                        # behind the first wq groups, not in front
                        nc.sync.dma_start(out=wo0, in_=woT[0])
                        nc.sync.dma_start(out=wg0, in_=wgT[0])
                    # denominators: one [1,512] bank per row chunk
                    rdenb = rdenp.tile([P, R], F16, name="rdenb")
                    for lq in range(LQ):
                        dpb = dpps.tile([1, 512], F32, name="dpb")
                        for mt in range(MT):
                            nc.tensor.matmul(
                                dpb, ones_t,
                                probs[:, mt, lq * 512:(lq + 1) * 512],
                                start=(mt == 0), stop=(mt == MT - 1))
                        rr = rrowp.tile([1, 512], F32, name="rr")
                        rh = rrowp.tile([1, 512], F16, name="rh")
                        nc.vector.reciprocal_approx_fast(rr, dpb)
                        nc.vector.tensor_copy(rh, rr)
                        nc.sync.dma_start(out=rd_scr[h, lq, :], in_=rh)
                        nc.sync.dma_start(
                            out=rdenb[:, lq * 512:(lq + 1) * 512],
                            in_=_bcast_ap(rd_scr[h, lq, :]))
                    # attention output, normalized at eviction
                    for lq in range(LQ):
                        ap_ = tmpps.tile([P, 512], F32, name="ap")
                        for mt in range(MT):
                            nc.tensor.matmul(
                                ap_, vmd[:, mt, h * P:(h + 1) * P],
                                probs[:, mt, lq * 512:(lq + 1) * 512],
                                start=(mt == 0), stop=(mt == MT - 1))
                        c = slice(lq * 512, (lq + 1) * 512)
                        nc.vector.tensor_mul(attn_all[:, h, c], ap_,
                                             rdenb[:, c])

            # ============== Phase D: O proj + gate, fused ==============
            with tc.tile_pool(name="wost", bufs=2) as wost, \
                 tc.tile_pool(name="wgst", bufs=2) as wgst, \
                 tc.tile_pool(name="gs", bufs=4) as gsp, \
                 tc.tile_pool(name="osb", bufs=2) as osbp, \
                 tc.tile_pool(name="ops", bufs=2, space="PSUM") as ops:
                for hog in range(NHG):
                    if hog == 0:
                        wo_t, wg_t = wo0, wg0
                    else:
                        wo_t = wost.tile([P, KT, 2 * P], F16, name="wo_t")
                        wg_t = wgst.tile([P, KT, 2 * P], F16, name="wg_t")
                        nc.sync.dma_start(out=wo_t, in_=woT[hog])
                        nc.sync.dma_start(out=wg_t, in_=wgT[hog])
                    for hh in range(2):
                        ho = hog * 2 + hh
                        for lqp in range(2):
                            op2 = [ops.tile([P, 512], F32, name=f"op{j}")
                                   for j in range(2)]
                            gp2 = [ops.tile([P, 512], F32, name=f"gp{j}")
                                   for j in range(2)]
                            for kt in range(KT):
                                for j in range(2):
                                    c = slice(lqp * 1024 + j * 512,
                                              lqp * 1024 + (j + 1) * 512)
                                    nc.tensor.matmul(
                                        op2[j],
                                        wo_t[:, kt, hh * P:(hh + 1) * P],
                                        attn_all[:, kt, c],
                                        start=(kt == 0), stop=(kt == KT - 1))
                                for j in range(2):
                                    c = slice(lqp * 1024 + j * 512,
                                              lqp * 1024 + (j + 1) * 512)
                                    nc.tensor.matmul(
                                        gp2[j],
                                        wg_t[:, kt, hh * P:(hh + 1) * P],
                                        xh[:, kt, c],
                                        start=(kt == 0), stop=(kt == KT - 1))
                            o_sb = osbp.tile([P, 1024], F32, name="o_sb")
                            for j in range(2):
                                gs = gsp.tile([P, 512], F16, name="gs")
                                nc.scalar.activation(
                                    gs, gp2[j],
                                    mybir.ActivationFunctionType.Sigmoid)
                                nc.vector.tensor_mul(
                                    o_sb[:, j * 512:(j + 1) * 512], op2[j], gs)
                            nc.sync.dma_start(
                                out=outT[ho * P:(ho + 1) * P,
                                         lqp * 1024:(lqp + 1) * 1024],
                                in_=o_sb)

    nc.compile()
    return nc


import numpy as np

_H, _NH, _HD, _M = 2048, 16, 128, 256
_B, _L = 4, 4096
_RPC = 2048          # rows per core
_NCORES = 8
_EPS = 1e-6

_nc_cache = [None]


def _tile_w(wT, width):
    """[in, out] f32 -> [n, P, KT, width] fp16, p-major for large DMA lines."""
    KT = wT.shape[0] // 128
    n = wT.shape[1] // width
    return np.ascontiguousarray(
        wT.reshape(KT, 128, n, width).transpose(2, 1, 0, 3).astype(np.float16))


def _prep_core(hs_slice, mem_b, mask_b, shared):
    inp = dict(shared)
    inp["xhT"] = np.ascontiguousarray(hs_slice.T.astype(np.float16))
    memt = mem_b.T.astype(np.float16)          # [H, M]
    inp["memT"] = np.ascontiguousarray(
        memt.reshape(_H // 128, 128, _M).transpose(1, 0, 2))
    maskb = np.where(mask_b, -LN256, -50.0).astype(np.float32)
    inp["maskb"] = np.ascontiguousarray(maskb.reshape(_M // 128, 128).T)
    return inp


def kernel(hidden_states, memory_tokens, memory_mask, norm_w,
           wq, wk, wv, wo, wg):
    import concourse.bacc as bacc

    hs = np.asarray(hidden_states, dtype=np.float32)
    mem = np.asarray(memory_tokens, dtype=np.float32)
    mask = np.asarray(memory_mask)
    norm_w = np.asarray(norm_w, dtype=np.float32)

    wq_n = (np.asarray(wq, dtype=np.float32) * norm_w[None, :]).T
    wg_n = (np.asarray(wg, dtype=np.float32) * norm_w[None, :]).T
    shared = {
        "wqT": _tile_w(np.ascontiguousarray(wq_n), 256),
        "wgT": _tile_w(np.ascontiguousarray(wg_n), 256),
        "woT": _tile_w(np.ascontiguousarray(np.asarray(wo, dtype=np.float32).T), 256),
        "wkT": _tile_w(np.ascontiguousarray(np.asarray(wk, dtype=np.float32).T), 1024),
        "wvT": _tile_w(np.ascontiguousarray(np.asarray(wv, dtype=np.float32).T), 512),
    }

    in_maps = []
    for c in range(_NCORES):
        b, half = c // 2, c % 2
        hs_slice = hs[b, half * _RPC:(half + 1) * _RPC, :]
        in_maps.append(_prep_core(hs_slice, mem[b], mask[b], shared))

    if _nc_cache[0] is None:
        nc = bacc.Bacc(None, target_bir_lowering=False, debug=False)
        build(nc, _H, _NH, _RPC, _M, eps=_EPS)
        _nc_cache[0] = nc
    nc = _nc_cache[0]

    import os
    trace = os.environ.get("KERNEL_TRACE") == "1"
    res = run_bass_kernel_spmd(nc, in_maps, core_ids=list(range(_NCORES)),
                               trace=trace)
    kernel.last_result = res

    out = np.empty((_B, _L, _H), dtype=np.float32)
    for c in range(_NCORES):
        b, half = c // 2, c % 2
        out[b, half * _RPC:(half + 1) * _RPC, :] = res.results[c]["outT"].T
    return out


# revision 30
# speedup vs baseline: 1.6966x; 1.0216x over previous
"""MemoryCrossAttention Trainium2 Bass kernel (fp16 full-rate rewrite).

8-core data-parallel over query rows: core c handles batch c//2, row-half
c%2 (2048 rows). All matmuls run in fp16 (full PE rate; fp32r is
throttled to half rate on TRN2 hardware). Weights and x are cast to fp16
on the host with p-major tiling so every weight group loads in one
large-line DMA. Everything stays resident in SBUF (no DRAM spills).
Emission order keeps the PE fed: K/V projections run first (they only
need the small memory-token tiles), the RMSNorm square-sum accumulates
behind them, and phase C is software-pipelined (Q-projection of head h+1
is emitted between the scores and denominator of head h so the exp/
reciprocal latency never stalls the PE). The gate projection is fused
into the O-projection with sigmoid applied at PSUM eviction.
"""
from concourse.bass_utils import run_bass_kernel_spmd


from contextlib import ExitStack

import concourse.bass as bass
import concourse.tile as tile
from concourse import mybir

F32 = mybir.dt.float32
F16 = mybir.dt.float16
P = 128
LN256 = 5.545177444479562


def _bcast_ap(row_ap):
    """[1, n] AP -> [128, n] partition-broadcast AP (stride 0)."""
    return bass.AP(tensor=row_ap.tensor, offset=row_ap.offset,
                   ap=[[0, P]] + row_ap.ap)


def build(nc, H, NH, R, M, eps=1e-6):
    HD = 128
    assert H == NH * HD
    KT = H // P           # contraction tiles (16)
    LQ = R // 512         # 512-wide row chunks (4)
    MT = M // P           # memory-token partition tiles (2)
    NHG = NH // 2         # 2-head weight groups (8)
    KH = 8                # heads per K-proj psum group
    NKG = NH // KH        # 2
    NVC = H // 512        # V d-chunks (4)
    scale = HD ** -0.5

    xhT = nc.dram_tensor("xhT", [H, R], F16, kind="ExternalInput")
    memT = nc.dram_tensor("memT", [P, KT, M], F16, kind="ExternalInput")
    maskb = nc.dram_tensor("maskb", [P, MT], F32, kind="ExternalInput")
    wqT = nc.dram_tensor("wqT", [NHG, P, KT, 2 * P], F16, kind="ExternalInput")
    wgT = nc.dram_tensor("wgT", [NHG, P, KT, 2 * P], F16, kind="ExternalInput")
    woT = nc.dram_tensor("woT", [NHG, P, KT, 2 * P], F16, kind="ExternalInput")
    wkT = nc.dram_tensor("wkT", [NKG, P, KT, KH * P], F16, kind="ExternalInput")
    wvT = nc.dram_tensor("wvT", [NVC, P, KT, 512], F16, kind="ExternalInput")
    outT = nc.dram_tensor("outT", [H, R], F32, kind="ExternalOutput")

    with tile.TileContext(nc) as tc, ExitStack() as ctx:
        dram = ctx.enter_context(tc.tile_pool(name="dram", bufs=1, space="DRAM"))
        rs_scr = dram.tile([LQ, 512], F16)
        rd_scr = dram.tile([NH, LQ, 512], F16)

        const = ctx.enter_context(tc.tile_pool(name="const", bufs=1))
        ones_t = const.tile([P, 1], F16)
        nc.vector.memset(ones_t, 1.0)
        eps_sb = const.tile([P, 1], F32)
        nc.vector.memset(eps_sb, eps)
        mask_sb = const.tile([P, MT], F32)
        nc.sync.dma_start(out=mask_sb, in_=maskb[:])

        # persistent activations
        xpool = ctx.enter_context(tc.tile_pool(name="x", bufs=1))
        xh = xpool.tile([P, KT, R], F16)          # x, then xn in place
        kvpool = ctx.enter_context(tc.tile_pool(name="kv", bufs=1))
        kT_all = kvpool.tile([P, NH, M], F16)     # [d, h, m]
        vmd = kvpool.tile([P, MT, H], F16)        # [m, mt, d_full]
        apool = ctx.enter_context(tc.tile_pool(name="attn", bufs=1))
        attn_all = apool.tile([P, NH, R], F16)    # [d, h, rows]

        # ============= Phase B: K/V from memory tokens =============
        # DMA issue order is the startup critical path: memory tokens and
        # K weights go first so the PE starts at ~5us; x tiles stream in
        # two batches between the K-weight chunks; V weights follow.
        with tc.tile_pool(name="memp", bufs=1) as memp:
            memh = memp.tile([P, KT, M], F16)
            nc.sync.dma_start(out=memh, in_=memT[:])

            with tc.tile_pool(name="wkst", bufs=3) as wkst, \
                 tc.tile_pool(name="kps", bufs=1, space="PSUM") as kps:
                KT2 = KT // 2
                wk_t = {}
                for kt in range(KT):
                    nc.gpsimd.dma_start(out=xh[:, kt, :],
                                        in_=xhT[kt * P:(kt + 1) * P, :])
                for kg, half in ((0, 0), (0, 1), (1, 0), (1, 1)):
                    wk_t[kg, half] = wkst.tile([P, KT2, KH * P], F16,
                                               name="wk_t")
                    nc.sync.dma_start(
                        out=wk_t[kg, half],
                        in_=wkT[kg][:, half * KT2:(half + 1) * KT2, :])
                for kg in range(NKG):
                    kpsum = [kps.tile([P, M], F32, name=f"kpsum{i}")
                             for i in range(KH)]
                    for half in range(2):
                        for k8 in range(KT2):
                            kt = half * KT2 + k8
                            for hh in range(KH):
                                nc.tensor.matmul(
                                    kpsum[hh],
                                    wk_t[kg, half][:, k8, hh * P:(hh + 1) * P],
                                    memh[:, kt, :],
                                    start=(kt == 0), stop=(kt == KT - 1))
                    for hh in range(KH):
                        nc.scalar.copy(kT_all[:, kg * KH + hh, :], kpsum[hh])

            with tc.tile_pool(name="wvst", bufs=2) as wvst, \
                 tc.tile_pool(name="vps", bufs=2, space="PSUM") as vps:
                for dc in range(NVC):
                    wv_t = wvst.tile([P, KT, 512], F16)
                    nc.gpsimd.dma_start(out=wv_t, in_=wvT[dc])
                    vpsum = [vps.tile([P, 512], F32, name=f"vpsum{i}")
                             for i in range(MT)]
                    for kt in range(KT):
                        for mt in range(MT):
                            nc.tensor.matmul(
                                vpsum[mt], memh[:, kt, mt * P:(mt + 1) * P],
                                wv_t[:, kt, :],
                                start=(kt == 0), stop=(kt == KT - 1))
                    for mt in range(MT):
                        nc.scalar.copy(
                            vmd[:, mt, dc * 512:(dc + 1) * 512], vpsum[mt])

        # ================= Phase A: rmsnorm (behind K/V) =================
        with tc.tile_pool(name="x2", bufs=2) as x2p, \
             tc.tile_pool(name="ssqp", bufs=1, space="PSUM") as ssqp, \
             tc.tile_pool(name="rsp", bufs=1) as rsp:
            ssq = [ssqp.tile([1, 512], F32, name=f"ssq{j}")
                   for j in range(LQ)]
            for kt in range(KT):
                x2 = x2p.tile([P, R], F16)
                nc.vector.tensor_mul(x2, xh[:, kt, :], xh[:, kt, :])
                for lq in range(LQ):
                    nc.tensor.matmul(
                        ssq[lq], ones_t,
                        x2[:, lq * 512:(lq + 1) * 512],
                        start=(kt == 0), stop=(kt == KT - 1))
            s_sb = rsp.tile([1, LQ, 512], F32)
            rs_sb = rsp.tile([1, LQ, 512], F32)
            rsh_sb = rsp.tile([1, LQ, 512], F16)
            rsb = rsp.tile([P, R], F16)
            for lq in range(LQ):
                nc.scalar.activation(
                    s_sb[:, lq, :], ssq[lq],
                    mybir.ActivationFunctionType.Sqrt,
                    bias=eps_sb[0:1, :], scale=1.0 / H)
                nc.vector.reciprocal_approx_fast(rs_sb[:, lq, :],
                                                 s_sb[:, lq, :])
                nc.vector.tensor_copy(rsh_sb[:, lq, :], rs_sb[:, lq, :])
                nc.sync.dma_start(out=rs_scr[lq, :], in_=rsh_sb[:, lq, :])
                nc.sync.dma_start(
                    out=rsb[:, lq * 512:(lq + 1) * 512],
                    in_=_bcast_ap(rs_scr[lq, :]))
            for lq in range(LQ):
                c = slice(lq * 512, (lq + 1) * 512)
                for kt in range(KT):
                    nc.vector.tensor_mul(xh[:, kt, c], xh[:, kt, c], rsb[:, c])

        # ====== preload O/gate group 0 so phase D starts instantly ======
        with tc.tile_pool(name="wod0", bufs=1) as w0p:
            wo0 = w0p.tile([P, KT, 2 * P], F16, name="wo0")
            wg0 = w0p.tile([P, KT, 2 * P], F16, name="wg0")

            # ===== Phase C: Q proj + attention, software-pipelined =====
            with tc.tile_pool(name="wqst", bufs=2) as wqst, \
                 tc.tile_pool(name="qh", bufs=2) as qhp, \
                 tc.tile_pool(name="probs", bufs=1) as probsp, \
                 tc.tile_pool(name="rden", bufs=1) as rdenp, \
                 tc.tile_pool(name="rrow", bufs=2) as rrowp, \
                 tc.tile_pool(name="qps", bufs=2, space="PSUM") as qps, \
                 tc.tile_pool(name="dpps", bufs=2, space="PSUM") as dpps, \
                 tc.tile_pool(name="tmpps", bufs=2, space="PSUM") as tmpps:
                wq_tiles = {}

                def qproj(h):
                    hg, hh = divmod(h, 2)
                    if hh == 0:
                        wq_t = wqst.tile([P, KT, 2 * P], F16, name="wq_t")
                        nc.sync.dma_start(out=wq_t, in_=wqT[hg])
                        wq_tiles[hg] = wq_t
                    wq_t = wq_tiles[hg]
                    qh = qhp.tile([P, R], F16, name="qh")
                    for lq in range(LQ):
                        qpsum = qps.tile([P, 512], F32, name="qpsum")
                        for kt in range(KT):
                            nc.tensor.matmul(
                                qpsum, wq_t[:, kt, hh * P:(hh + 1) * P],
                                xh[:, kt, lq * 512:(lq + 1) * 512],
                                start=(kt == 0), stop=(kt == KT - 1))
                        nc.scalar.copy(qh[:, lq * 512:(lq + 1) * 512], qpsum)
                    return qh

                qh_cur = qproj(0)
                for h in range(NH):
                    # scores -> probs (exp with mask bias, /256 folded in)
                    probs = probsp.tile([P, MT, R], F16, name="probs")
                    for mt in range(MT):
                        for lq in range(LQ):
                            sp = tmpps.tile([P, 512], F32, name="sp")
                            nc.tensor.matmul(
                                sp, kT_all[:, h, mt * P:(mt + 1) * P],
                                qh_cur[:, lq * 512:(lq + 1) * 512],
                                start=True, stop=True)
                            nc.scalar.activation(
                                probs[:, mt, lq * 512:(lq + 1) * 512], sp,
                                mybir.ActivationFunctionType.Exp,
                                bias=mask_sb[:, mt:mt + 1], scale=scale)
                    # Q projection of the NEXT head hides exp/recip latency
                    if h + 1 < NH:
                        qh_cur = qproj(h + 1)
                    if h == 1:
                        # D group-0 weights: issued here so they queue
                        # behind the first wq groups, not in front
                        nc.sync.dma_start(out=wo0, in_=woT[0])
                        nc.sync.dma_start(out=wg0, in_=wgT[0])
                    # denominators: one [1,512] bank per row chunk
                    rdenb = rdenp.tile([P, R], F16, name="rdenb")
                    for lq in range(LQ):
                        dpb = dpps.tile([1, 512], F32, name="dpb")
                        for mt in range(MT):
                            nc.tensor.matmul(
                                dpb, ones_t,
                                probs[:, mt, lq * 512:(lq + 1) * 512],
                                start=(mt == 0), stop=(mt == MT - 1))
                        rr = rrowp.tile([1, 512], F32, name="rr")
                        rh = rrowp.tile([1, 512], F16, name="rh")
                        nc.vector.reciprocal_approx_fast(rr, dpb)
                        nc.vector.tensor_copy(rh, rr)
                        nc.sync.dma_start(out=rd_scr[h, lq, :], in_=rh)
                        nc.sync.dma_start(
                            out=rdenb[:, lq * 512:(lq + 1) * 512],
                            in_=_bcast_ap(rd_scr[h, lq, :]))
                    # attention output, normalized at eviction
                    for lq in range(LQ):
                        ap_ = tmpps.tile([P, 512], F32, name="ap")
                        for mt in range(MT):
                            nc.tensor.matmul(
                                ap_, vmd[:, mt, h * P:(h + 1) * P],
                                probs[:, mt, lq * 512:(lq + 1) * 512],
                                start=(mt == 0), stop=(mt == MT - 1))
                        c = slice(lq * 512, (lq + 1) * 512)
                        nc.vector.tensor_mul(attn_all[:, h, c], ap_,
                                             rdenb[:, c])

            # ============== Phase D: O proj + gate, fused ==============
            with tc.tile_pool(name="wost", bufs=2) as wost, \
                 tc.tile_pool(name="wgst", bufs=2) as wgst, \
                 tc.tile_pool(name="gs", bufs=4) as gsp, \
                 tc.tile_pool(name="osb", bufs=2) as osbp, \
                 tc.tile_pool(name="ops", bufs=2, space="PSUM") as ops:
                for hog in range(NHG):
                    if hog == 0:
                        wo_t, wg_t = wo0, wg0
                    else:
                        wo_t = wost.tile([P, KT, 2 * P], F16, name="wo_t")
                        wg_t = wgst.tile([P, KT, 2 * P], F16, name="wg_t")
                        nc.sync.dma_start(out=wo_t, in_=woT[hog])
                        nc.sync.dma_start(out=wg_t, in_=wgT[hog])
                    for hh in range(2):
                        ho = hog * 2 + hh
                        for lqp in range(2):
                            op2 = [ops.tile([P, 512], F32, name=f"op{j}")
                                   for j in range(2)]
                            gp2 = [ops.tile([P, 512], F32, name=f"gp{j}")
                                   for j in range(2)]
                            for kt in range(KT):
                                for j in range(2):
                                    c = slice(lqp * 1024 + j * 512,
                                              lqp * 1024 + (j + 1) * 512)
                                    nc.tensor.matmul(
                                        op2[j],
                                        wo_t[:, kt, hh * P:(hh + 1) * P],
                                        attn_all[:, kt, c],
                                        start=(kt == 0), stop=(kt == KT - 1))
                                for j in range(2):
                                    c = slice(lqp * 1024 + j * 512,
                                              lqp * 1024 + (j + 1) * 512)
                                    nc.tensor.matmul(
                                        gp2[j],
                                        wg_t[:, kt, hh * P:(hh + 1) * P],
                                        xh[:, kt, c],
                                        start=(kt == 0), stop=(kt == KT - 1))
                            o_sb = osbp.tile([P, 1024], F32, name="o_sb")
                            for j in range(2):
                                gs = gsp.tile([P, 512], F16, name="gs")
                                nc.scalar.activation(
                                    gs, gp2[j],
                                    mybir.ActivationFunctionType.Sigmoid)
                                nc.vector.tensor_mul(
                                    o_sb[:, j * 512:(j + 1) * 512], op2[j], gs)
                            nc.sync.dma_start(
                                out=outT[ho * P:(ho + 1) * P,
                                         lqp * 1024:(lqp + 1) * 1024],
                                in_=o_sb)

    nc.compile()
    return nc


import numpy as np

_H, _NH, _HD, _M = 2048, 16, 128, 256
_B, _L = 4, 4096
_RPC = 2048          # rows per core
_NCORES = 8
_EPS = 1e-6

_nc_cache = [None]


def _tile_w(wT, width):
    """[in, out] f32 -> [n, P, KT, width] fp16, p-major for large DMA lines."""
    KT = wT.shape[0] // 128
    n = wT.shape[1] // width
    return np.ascontiguousarray(
        wT.reshape(KT, 128, n, width).transpose(2, 1, 0, 3).astype(np.float16))


def _prep_core(hs_slice, mem_b, mask_b, shared):
    inp = dict(shared)
    inp["xhT"] = np.ascontiguousarray(hs_slice.T.astype(np.float16))
    memt = mem_b.T.astype(np.float16)          # [H, M]
    inp["memT"] = np.ascontiguousarray(
        memt.reshape(_H // 128, 128, _M).transpose(1, 0, 2))
    maskb = np.where(mask_b, -LN256, -50.0).astype(np.float32)
    inp["maskb"] = np.ascontiguousarray(maskb.reshape(_M // 128, 128).T)
    return inp


def kernel(hidden_states, memory_tokens, memory_mask, norm_w,
           wq, wk, wv, wo, wg):
    import concourse.bacc as bacc

    hs = np.asarray(hidden_states, dtype=np.float32)
    mem = np.asarray(memory_tokens, dtype=np.float32)
    mask = np.asarray(memory_mask)
    norm_w = np.asarray(norm_w, dtype=np.float32)

    wq_n = (np.asarray(wq, dtype=np.float32) * norm_w[None, :]).T
    wg_n = (np.asarray(wg, dtype=np.float32) * norm_w[None, :]).T
    shared = {
        "wqT": _tile_w(np.ascontiguousarray(wq_n), 256),
        "wgT": _tile_w(np.ascontiguousarray(wg_n), 256),
        "woT": _tile_w(np.ascontiguousarray(np.asarray(wo, dtype=np.float32).T), 256),
        "wkT": _tile_w(np.ascontiguousarray(np.asarray(wk, dtype=np.float32).T), 1024),
        "wvT": _tile_w(np.ascontiguousarray(np.asarray(wv, dtype=np.float32).T), 512),
    }

    in_maps = []
    for c in range(_NCORES):
        b, half = c // 2, c % 2
        hs_slice = hs[b, half * _RPC:(half + 1) * _RPC, :]
        in_maps.append(_prep_core(hs_slice, mem[b], mask[b], shared))

    if _nc_cache[0] is None:
        nc = bacc.Bacc(None, target_bir_lowering=False, debug=False)
        build(nc, _H, _NH, _RPC, _M, eps=_EPS)
        _nc_cache[0] = nc
    nc = _nc_cache[0]

    import os
    trace = os.environ.get("KERNEL_TRACE") == "1"
    res = run_bass_kernel_spmd(nc, in_maps, core_ids=list(range(_NCORES)),
                               trace=trace)
    kernel.last_result = res

    out = np.empty((_B, _L, _H), dtype=np.float32)
    for c in range(_NCORES):
        b, half = c // 2, c % 2
        out[b, half * _RPC:(half + 1) * _RPC, :] = res.results[c]["outT"].T
    return out
